# revision 1
# baseline (speedup 1.0000x reference)
"""Trainium2 Bass kernel for time-decayed causal KNN retrieval + fusion scoring.

Math (reference):
  sim_t[i,j] = cos(q_i, p_j) * exp(-l*|ti-tj|)
  masked     = causal(tj < ti) ? sim_t : -inf   (rows with no causal keep sim_t)
  top-7 by masked value -> cross-attn fusion -> deviation score  [Bq]

Strategy (8 NeuronCores, pool-sharded):
  * For causal pairs exp(-l*|ti-tj|) = exp(-l*ti)*exp(l*tj): fold the decay
    and the L2 norms into the matmul operands on the host (non-causal pairs
    get a wrong decay but are masked out on the host anyway).
  * Sort pool by time, shard round-robin across 8 cores (8192 items/core,
    balanced); sort queries by time. The causal mask becomes a column-prefix
    per row; only the causal prefix of 512-col chunks is computed per
    128-query tile (~45% of the full slab skipped).
  * Device per core: fp32r matmuls (full PE rate) -> PSUM; ACT stages the
    even/odd column halves to SBUF as bf16; DVE runs a 3-level pairwise-max
    tree (bf16 2x mode) producing per-row maxima of 8-element windows; the
    window-max band (<=1024 windows/tile) is DMA'd out. No top-k on device.
  * Host: applies the exact causal window kill, takes each row's global
    top-7-by-window-max threshold (bf16 monotonicity makes the containment
    of the true top-7 elements exact), rescores the selected ~10 windows'
    80 columns exactly in float64, selects top-7 with reference tie
    semantics, and computes the softmax fusion + anomaly score (trivial
    FLOPs).

Window layout (64-col blocks, contiguous APs at every tree level):
  the device tree makes window j of a 64-col block cover device columns
  2j + {0,1,16,17,32,33,48,49}; the host pre-scatters each shard's columns
  within every 64-block (_block_perm) so that device window w corresponds to
  the time-contiguous shard columns [8w, 8w+8) — the causal boundary then
  touches only one window per core and window kill is a simple prefix test.
"""

import numpy as np

BQ, BN, H, K = 2048, 65536, 256, 7
NCORES = 8
LAMBDA = 0.1
GAMMA, DELTA = 0.5, 0.5
EPS = 1e-12
COS_EPS = 1e-8
CHUNK = 512
SHARD = BN // NCORES  # 8192
QTILE = 128
NTILES = BQ // QTILE  # 16
NWIN_MAX = SHARD // 8  # 1024
WIN_OFFS = np.array([0, 1, 16, 17, 32, 33, 48, 49], dtype=np.int64)
MARGIN = 8  # extra windows beyond K in the host threshold selection
MAXW_ROW = 48  # cap on host-selected windows per row before full fallback

_PROGRAM_CACHE = {}


def _block_perm():
    """Within-64-block column permutation applied to each shard on the host.

    The device max-tree (contiguous APs) makes window j of a 64-col block
    cover device columns 2j + {0,1,16,17,32,33,48,49}. Placing time-rank
    8j + o at device position 2j + offs[o] makes device window w equal the
    time-contiguous shard columns [8w, 8w+8), so the causal boundary touches
    only one window per core.
    """
    d = np.arange(64, dtype=np.int64)
    o = 2 * (d // 16) + (d % 2)
    return 8 * ((d % 16) // 2) + o  # P[d] = time-rank placed at device col d


def _build_program(pt_list, reps=1):
    import concourse.bacc as bacc
    import concourse.mybir as mybir
    import concourse.tile as tile

    f32r = mybir.dt.float32r
    f32 = mybir.dt.float32
    bf16 = mybir.dt.bfloat16

    nc = bacc.Bacc("TRN2", target_bir_lowering=False, debug=False)

    qT_d = nc.dram_tensor("qT", [H, BQ], f32r, kind="ExternalInput")
    pT_d = nc.dram_tensor("pT", [H, SHARD], f32r, kind="ExternalInput")
    wb_d = nc.dram_tensor("wb", [BQ, NWIN_MAX], bf16, kind="ExternalOutput")

    GRP = 2  # chunks per PSUM tile (2 banks); 4 tiles in flight = 8 banks
    GW = GRP * CHUNK
    MAXOP = mybir.AluOpType.max

    with tile.TileContext(nc) as tc:
        with (
            tc.tile_pool(name="resident", bufs=1) as resp,
            tc.tile_pool(name="wband", bufs=3) as wbandp,
            tc.tile_pool(name="lvl", bufs=4) as lvlp,
            tc.tile_pool(name="psum", bufs=4, space="PSUM") as psump,
        ):
          for _rep in range(reps):
            p_sb = []
            q_sb = []
            for h in range(2):
                q_sb.append(resp.tile([128, BQ], f32r, tag=f"q{h}", name=f"q{h}"))
                p_sb.append(resp.tile([128, SHARD], f32r, tag=f"p{h}", name=f"p{h}"))
            # process the smallest tile last to shrink the kernel-tail drain;
            # queries via sync-engine DMA, pool pieces via gpsimd (separate
            # trigger stream) so the first matmuls start early
            tile_order = list(range(1, NTILES)) + [0]
            for h in range(2):
                nc.sync.dma_start(q_sb[h][:], qT_d[h * 128 : (h + 1) * 128, :])
            for c0 in range(0, SHARD, GW):
                for h in range(2):
                    nc.gpsimd.dma_start(
                        p_sb[h][:, c0 : c0 + GW],
                        pT_d[h * 128 : (h + 1) * 128, c0 : c0 + GW],
                    )

            for t in tile_order:
                pt_len = pt_list[t]
                nchunks = pt_len // CHUNK
                nwin = pt_len // 8
                wband = wbandp.tile([QTILE, nwin], bf16, tag="wband")

                for g in range(0, nchunks, GRP):
                    ge = min(g + GRP, nchunks)
                    gw = (ge - g) * CHUNK
                    ps = psump.tile([QTILE, gw], f32, tag="ps", name=f"ps{t}_{g}")
                    # h-outer: load each stationary q-tile once per group
                    for h in range(2):
                        for j, c in enumerate(range(g, ge)):
                            nc.tensor.matmul(
                                ps[:, j * CHUNK : (j + 1) * CHUNK],
                                q_sb[h][:, t * QTILE : (t + 1) * QTILE],
                                p_sb[h][:, c * CHUNK : (c + 1) * CHUNK],
                                start=(h == 0),
                                stop=(h == 1),
                                skip_group_check=True,
                            )
                    # level1: ACT stages the even half to SBUF as bf16; DVE
                    # maxes it against the odd half. Only one PSUM operand is
                    # allowed per DVE op, so on alternating groups ACT stages
                    # BOTH halves and DVE runs in bf16 2x mode — balances the
                    # ACT/DVE load.
                    w1a = lvlp.tile([QTILE, GW // 2], bf16, tag="w1a")
                    nc.scalar.copy(out=w1a[:, : gw // 2], in_=ps[:, 0:gw:2])
                    w1 = lvlp.tile([QTILE, GW // 2], bf16, tag="w1")
                    if (g // GRP) % 2 == 0:
                        odd_src = ps[:, 1:gw:2]
                    else:
                        w1b = lvlp.tile([QTILE, GW // 2], bf16, tag="w1b")
                        nc.scalar.copy(out=w1b[:, : gw // 2], in_=ps[:, 1:gw:2])
                        odd_src = w1b[:, : gw // 2]
                    nc.vector.tensor_tensor(
                        out=w1[:, : gw // 2],
                        in0=w1a[:, : gw // 2],
                        in1=odd_src,
                        op=MAXOP,
                    )
                    w1r = w1[:, : gw // 2].rearrange("p (b x) -> p b x", x=32)
                    w2 = lvlp.tile([QTILE, GW // 4], bf16, tag="w2")
                    w2r = w2[:, : gw // 4].rearrange("p (b x) -> p b x", x=16)
                    nc.vector.tensor_tensor(
                        out=w2r[:, :, :],
                        in0=w1r[:, :, 0:16],
                        in1=w1r[:, :, 16:32],
                        op=MAXOP,
                    )
                    wbr = wband[
                        :, g * (CHUNK // 8) : g * (CHUNK // 8) + gw // 8
                    ].rearrange("p (b x) -> p b x", x=8)
                    nc.vector.tensor_tensor(
                        out=wbr[:, :, :],
                        in0=w2r[:, :, 0:8],
                        in1=w2r[:, :, 8:16],
                        op=MAXOP,
                    )

                nc.sync.dma_start(
                    wb_d[t * QTILE : (t + 1) * QTILE, :nwin], wband[:]
                )

    nc.compile()
    return nc


def _prepare(query_emb, query_time, pool_emb, pool_time):
    """Host preprocessing: fold norms+decay into operands, sort, shard."""
    q = query_emb.astype(np.float64)
    p = pool_emb.astype(np.float64)
    qt = query_time.astype(np.float64)
    pt = pool_time.astype(np.float64)

    qnorm = np.linalg.norm(q, axis=1)
    pnorm = np.linalg.norm(p, axis=1)
    qs = (q / np.maximum(qnorm, EPS)[:, None]) * np.exp(-LAMBDA * qt)[:, None]
    ps = (p / np.maximum(pnorm, EPS)[:, None]) * np.exp(LAMBDA * pt)[:, None]

    pperm = np.argsort(pool_time, kind="stable")
    qperm = np.argsort(query_time, kind="stable")
    ps_sorted = ps[pperm]
    pt_sorted = pool_time[pperm]
    qs_sorted = qs[qperm]
    qt_sorted = query_time[qperm]

    # scatter each shard's columns within every 64-block so device windows
    # are time-contiguous (see _block_perm)
    scatter = (
        np.arange(0, SHARD, 64)[:, None] + _block_perm()[None, :]
    ).reshape(-1)
    shard_emb = [
        np.ascontiguousarray(ps_sorted[k::NCORES][scatter].T, dtype=np.float32)
        for k in range(NCORES)
    ]
    shard_times = [pt_sorted[k::NCORES] for k in range(NCORES)]
    # exact count of shard items with tj < ti (strict), per core per sorted query
    ci = np.stack(
        [np.searchsorted(shard_times[k], qt_sorted, side="left") for k in range(NCORES)]
    ).astype(np.int64)  # [8, 2048]

    qT = np.ascontiguousarray(qs_sorted.T, dtype=np.float32)  # [256, 2048]
    return qT, shard_emb, ci, pperm, qperm


def _pt_list(ci):
    ci_tiles = ci.reshape(NCORES, NTILES, QTILE)
    maxci = ci_tiles.max(axis=0).max(axis=1)  # [NTILES]
    return np.clip(
        np.ceil(maxci / CHUNK).astype(np.int64) * CHUNK, CHUNK, SHARD
    ).tolist()


def _core_in_map(qT, shard_emb, k):
    return {"qT": qT, "pT": shard_emb[k]}


def _device_windows(qT, shard_emb, ci):
    """Run the Bass kernel; return per-core window-max bands [8, 2048, 1024]."""
    from concourse.bass_utils import run_bass_kernel_spmd

    pt_list = _pt_list(ci)
    key = tuple(pt_list)
    if key not in _PROGRAM_CACHE:
        _PROGRAM_CACHE.clear()
        _PROGRAM_CACHE[key] = _build_program(pt_list)
    nc = _PROGRAM_CACHE[key]

    in_maps = [_core_in_map(qT, shard_emb, k) for k in range(NCORES)]
    res = run_bass_kernel_spmd(nc, in_maps, core_ids=list(range(NCORES)))
    wb = np.stack(
        [res.results[k]["wb"].astype(np.float32) for k in range(NCORES)]
    )  # [8, 2048, 1024]
    return wb, pt_list


def _merge_and_score(
    wb, pt_list, ci, pperm, qperm, query_emb, query_time, pool_emb, pool_time
):
    """Select candidate windows by global threshold, rescore exactly, score."""
    nq = BQ
    wmin = 8 * np.arange(NWIN_MAX, dtype=np.int64)  # window min time-col

    # validity: window exists for the row's tile and contains >=1 causal col
    nwin_row = (np.asarray(pt_list, dtype=np.int64) // 8)[
        np.repeat(np.arange(NTILES), QTILE)
    ]  # [2048]
    exists = np.arange(NWIN_MAX)[None, :] < nwin_row[:, None]  # [2048, 1024]
    wbm = np.where(
        exists[None, :, :] & (wmin[None, None, :] < ci[:, :, None]),
        wb,
        -np.inf,
    )  # [8, 2048, 1024]

    flat = np.transpose(wbm, (1, 0, 2)).reshape(nq, NCORES * NWIN_MAX)
    KM = K + MARGIN
    kth = np.partition(flat, -KM, axis=1)[:, -KM]  # (K+MARGIN)-th largest
    # relax by ~2 bf16 ulps to absorb fp32r/bf16 rounding asymmetries
    kth = kth - (np.abs(kth) * 2.0**-7 + 1e-6)
    # rows with fewer than K+MARGIN valid windows: select all valid ones
    thr = np.where(np.isfinite(kth), kth, -1.0e38)
    sel = flat >= thr[:, None]
    nsel = sel.sum(axis=1)

    rows, wcols = np.nonzero(sel)
    core = wcols // NWIN_MAX
    w = wcols % NWIN_MAX
    # candidate columns: global time-sorted position -> original pool index
    cols_shard = (8 * w)[:, None] + np.arange(8)[None, :]  # [nsel, 8]
    sorted_pos = cols_shard * NCORES + core[:, None]
    orig = pperm[sorted_pos]  # [nsel_total, 8] original pool rows

    # exact rescore in float64
    q64 = query_emb.astype(np.float64)
    qn64 = q64 / np.maximum(np.linalg.norm(q64, axis=1), EPS)[:, None]
    pnorm = np.linalg.norm(pool_emb.astype(np.float64), axis=1)
    oi_rows = qperm[rows]  # original query row per selected window
    n_ent = rows.shape[0]
    sims = np.empty((n_ent, 8), dtype=np.float64)
    causal = np.empty((n_ent, 8), dtype=bool)
    BLK = 65536
    for b in range(0, n_ent, BLK):
        sl = slice(b, b + BLK)
        emb = pool_emb[orig[sl]].astype(np.float64)  # [blk, 8, 256]
        pn = np.maximum(pnorm[orig[sl]], EPS)
        dots = np.einsum("nh,nch->nc", qn64[oi_rows[sl]], emb) / pn
        tdiff = np.abs(
            query_time[oi_rows[sl]].astype(np.float64)[:, None]
            - pool_time[orig[sl]].astype(np.float64)
        )
        sims[sl] = dots * np.exp(-LAMBDA * tdiff)
        causal[sl] = pool_time[orig[sl]] < query_time[oi_rows[sl]][:, None]

    # scatter into dense per-row candidate arrays
    maxw = min(int(nsel.max()), MAXW_ROW)
    slot = np.zeros(n_ent, dtype=np.int64)
    if n_ent:
        # rows is sorted; position of each entry within its row
        row_start = np.searchsorted(rows, np.arange(nq), side="left")
        slot = np.arange(n_ent) - row_start[rows]
    keep = slot < MAXW_ROW
    dsims = np.full((nq, maxw * 8), -np.inf)
    dorig = np.zeros((nq, maxw * 8), dtype=np.int64)
    rk = rows[keep]
    sk = slot[keep]
    for o in range(8):
        dsims[rk, sk * 8 + o] = np.where(causal[keep, o], sims[keep, o], -np.inf)
        dorig[rk, sk * 8 + o] = orig[keep, o]

    order2 = np.lexsort((dorig, -dsims), axis=1)[:, :K]
    top_idx = np.take_along_axis(dorig, order2, axis=1)
    nvalid_row = np.isfinite(np.take_along_axis(dsims, order2, axis=1)).sum(axis=1)

    # rows needing the exact slow path
    pt_min = pool_time.min()
    n_causal_global = np.searchsorted(
        np.sort(pool_time), query_time[qperm], side="left"
    )
    fix_rows = np.nonzero(
        (query_time[qperm] <= pt_min)
        | (np.minimum(n_causal_global, K) > nvalid_row)
        | (n_causal_global < K)
        | (nsel > MAXW_ROW)
    )[0]
    if len(fix_rows):
        pn_all = pool_emb.astype(np.float64) / np.maximum(pnorm, EPS)[:, None]
    for i in fix_rows:
        oi = qperm[i]
        ti = query_time[oi]
        sims_all = (pn_all @ qn64[oi]) * np.exp(
            -LAMBDA * np.abs(float(ti) - pool_time.astype(np.float64))
        )
        if ti <= pt_min:
            # row_all_inf: reference keeps unmasked decayed sims
            top_idx[i] = np.argsort(-sims_all, kind="stable")[:K]
            continue
        causal_all = pool_time < ti
        c = int(causal_all.sum())
        masked_all = np.where(causal_all, sims_all, -np.inf)
        picks = list(np.argsort(-masked_all, kind="stable")[: min(c, K)])
        # pad like jax.lax.top_k over -inf ties: lowest non-causal original idx
        j = 0
        while len(picks) < K:
            if not causal_all[j]:
                picks.append(j)
            j += 1
        top_idx[i] = np.array(picks, dtype=np.int64)

    # fusion + score in float64 (reference is f32; fp64 is strictly closer)
    q = query_emb.astype(np.float64)[qperm]  # sorted-query order
    retrieved = pool_emb.astype(np.float64)[top_idx]  # [2048, 7, 256]
    scale = float(H) ** -0.5
    logits = np.einsum("bh,bkh->bk", q, retrieved) * scale
    logits -= logits.max(axis=1, keepdims=True)
    e = np.exp(logits)
    attn = e / e.sum(axis=1, keepdims=True)
    fused = np.einsum("bk,bkh->bh", attn, retrieved)

    qn2 = np.linalg.norm(q, axis=1)
    fn2 = np.linalg.norm(fused, axis=1)
    cos = np.sum(q * fused, axis=1) / np.maximum(qn2 * fn2, COS_EPS)
    l2 = np.linalg.norm(q - fused, axis=1)
    score_sorted = GAMMA * (1.0 - cos) + DELTA * l2

    out = np.zeros(nq, dtype=np.float32)
    out[qperm] = score_sorted.astype(np.float32)
    return out


def kernel(query_emb, query_time, pool_emb, pool_time):
    query_emb = np.asarray(query_emb, dtype=np.float32)
    query_time = np.asarray(query_time, dtype=np.float32)
    pool_emb = np.asarray(pool_emb, dtype=np.float32)
    pool_time = np.asarray(pool_time, dtype=np.float32)

    qT, shard_emb, ci, pperm, qperm = _prepare(
        query_emb, query_time, pool_emb, pool_time
    )
    wb, pt_list = _device_windows(qT, shard_emb, ci)
    return _merge_and_score(
        wb, pt_list, ci, pperm, qperm, query_emb, query_time, pool_emb, pool_time
    )



# revision 3
# speedup vs baseline: 1.1264x; 1.1264x over previous
"""Trainium2 Bass kernel: fp8 DoubleRow matmul + 3-lane PSUM drain.

Device math per core (pool shard 8192, queries 2048, H=256):
  sims (scaled by 64*64) via ONE fp8e4m3 DoubleRow matmul per 512-col chunk
  (K=256 contraction in a single instruction at 2x PE rate) -> PSUM fp32.
  Window maxima (8 time-contiguous cols per window) extracted per 1024-col
  group by one of two drain lanes, chosen per (tile, group) by a greedy
  HW-cost balance (_type_map, shared with the host):
    type D: DVE tensor_reduce(max, x=8) directly from PSUM -> bf16 -> DMA out
    type R: ACT copies PSUM->SBUF bf16; Pool-issued DMA ships the raw block
            to HBM; the HOST computes those window maxima (cheap numpy)
  Matmul/LDW runs on PE, staging on ACT, reduces on DVE, raw DMA issue on
  GPSIMD, wb/p8 DMA on SP -> all five engine queues carry part of the drain.

Host: exact causal window-kill + global threshold with fp8-calibrated relax,
exact rescore of selected windows, reference-tie top-7, fusion + score.
"""

import numpy as np

BQ, BN, H, K = 2048, 65536, 256, 7
NCORES = 8
LAMBDA = 0.1
GAMMA, DELTA = 0.5, 0.5
EPS = 1e-12
COS_EPS = 1e-8
CHUNK = 512
GRP_COLS = 1024  # drain group = 2 chunks = 2 PSUM banks
SHARD = BN // NCORES  # 8192
QTILE = 128
NTILES = BQ // QTILE  # 16
NWIN_MAX = SHARD // 8  # 1024
SCALE = 64.0  # operand scale; sims scaled by SCALE^2 = 4096
MARGIN = 2
MAXW_ROW = 192
# fp8 quantization abs-error bound on scaled sims (calibrated: max ~54 over
# 16.7M-pair sample; containment needs 2*eps_max)
RELAX_ABS = 130.0
NGROUPS_MAX = SHARD // GRP_COLS  # 8
# fixed drain type per global group index:
#   D: DVE tensor_reduce window-max directly from PSUM -> wband
#   R: ACT stages PSUM->SBUF bf16; raw block DMA'd out; host window-maxes
# Tuned so DVE (D groups) and ACT (R stages) are balanced.
NR = 6  # raw output regions (max R groups per tile)
TILE_ORDER = list(range(1, NTILES)) + [0]
# greedy per-(tile, group) engine-cost model (ns). HW-calibrated: DVE
# tensor_reduce measured ~1.44x the CoreSim cost on hardware; ACT stage and
# Pool DMA issue track CoreSim.
D_COST = lambda gw: 1.68 * gw + 100.0  # noqa: E731  (DVE reduce, HW)
A_COST = lambda gw: 1.081 * gw + 75.0  # noqa: E731  (ACT stage)
P_COST = lambda gw: 0.77 * gw + 36.0   # noqa: E731  (Pool DMA issue)


def _type_map(pt_list):
    """Deterministic per-(tile, group) D/R assignment balancing DVE vs
    ACT/Pool load in emission order. Returns {t: [types...]}, and the raw
    region index per (t, g) = rank of g among tile t's R groups."""
    l_dve = 0.0
    l_act = 0.0
    l_pool = 1600.0  # q8 DMA issue
    tmap = {}
    for t in TILE_ORDER:
        pt_len = pt_list[t]
        ng = (pt_len + GRP_COLS - 1) // GRP_COLS
        types = []
        nr = 0
        for g in range(ng):
            gw = min(GRP_COLS, pt_len - g * GRP_COLS)
            cost_d = D_COST(gw)
            cost_a = A_COST(gw)
            cost_p = P_COST(gw)
            as_d = max(l_dve + cost_d, l_act, l_pool)
            as_r = max(l_dve, l_act + cost_a, l_pool + cost_p)
            if nr >= NR or as_d <= as_r:
                types.append("D")
                l_dve += cost_d
            else:
                types.append("R")
                l_act += cost_a
                l_pool += cost_p
                nr += 1
        tmap[t] = types
    return tmap

_PROGRAM_CACHE = {}


def _group_perm(gtype):
    """Within-group (1024 cols) permutation: P[d] = time-rank placed at
    device column d, such that window w covers time-ranks [8w, 8w+8).
    Both D (device tensor_reduce over 8 consecutive cols) and R (host
    max over 8 consecutive raw cols) use the identity layout."""
    return np.arange(GRP_COLS, dtype=np.int64)


def _build_program(pt_list, reps=1):
    import concourse.bacc as bacc
    import concourse.mybir as mybir
    import concourse.tile as tile

    f8 = mybir.dt.float8e4
    f32 = mybir.dt.float32
    bf16 = mybir.dt.bfloat16
    MAXOP = mybir.AluOpType.max
    DR = mybir.MatmulPerfMode.DoubleRow

    nc = bacc.Bacc("TRN2", target_bir_lowering=False, debug=False)

    q_d = nc.dram_tensor("q8", [128, 2 * BQ], f8, kind="ExternalInput")
    p_d = nc.dram_tensor("p8", [128, 2 * SHARD], f8, kind="ExternalInput")
    wb_d = nc.dram_tensor("wb", [BQ, NWIN_MAX], bf16, kind="ExternalOutput")
    raw_d = nc.dram_tensor(
        "raw", [BQ, NR * GRP_COLS], bf16, kind="ExternalOutput"
    )

    with tile.TileContext(nc) as tc:
        with (
            tc.tile_pool(name="resident", bufs=2) as resp,
            tc.tile_pool(name="wband", bufs=4) as wbandp,
            tc.tile_pool(name="stg", bufs=4) as stgp,
            tc.tile_pool(name="psum", bufs=4, space="PSUM") as psump,
        ):
          for _rep in range(reps):
            q_sb = resp.tile([128, 2 * BQ], f8, tag="q8", name="q8")
            p_sb = resp.tile([128, 2 * SHARD], f8, tag="p8", name="p8")
            # q8 via gpsimd (idle at start) so SP can stream p8 in parallel;
            # p8 pieces interleave the two ksub halves (a matmul needs BOTH
            # halves of its column range) with small pieces first so the
            # first tile's matmuls start ASAP
            for q0, q1 in ((128, 512), (512, 2048), (0, 128)):
                for s in range(2):
                    nc.gpsimd.dma_start(
                        q_sb[:, s * BQ + q0 : s * BQ + q1],
                        q_d[:, s * BQ + q0 : s * BQ + q1],
                    )
            for c0, c1 in ((0, 1024), (1024, 3072), (3072, 8192)):
                for s in range(2):
                    nc.sync.dma_start(
                        p_sb[:, s * SHARD + c0 : s * SHARD + c1],
                        p_d[:, s * SHARD + c0 : s * SHARD + c1],
                    )
            q3 = q_sb[:].rearrange("p (s q) -> p s q", s=2)  # [128,2,2048]
            p3 = p_sb[:].rearrange("p (s c) -> p s c", s=2)  # [128,2,8192]

            tmap = _type_map(pt_list)
            for t in TILE_ORDER:
                pt_len = pt_list[t]
                types = tmap[t]
                ng = len(types)
                r_of = {}
                for g in range(ng):
                    if types[g] == "R":
                        r_of[g] = len(r_of)

                # emission order alternates D and R groups so neither DVE
                # nor ACT waits through a burst of the other type's matmuls
                d_present = [g for g in range(ng) if types[g] == "D"]
                r_present = [g for g in range(ng) if types[g] == "R"]
                order = []
                for i in range(max(len(d_present), len(r_present))):
                    if i < len(d_present):
                        order.append(d_present[i])
                    if i < len(r_present):
                        order.append(r_present[i])
                for g in order:
                    gw = min(GRP_COLS, pt_len - g * GRP_COLS)
                    gtype = types[g]
                    ps = psump.tile([QTILE, gw], f32, tag="ps",
                                    name=f"ps{t}_{g}")
                    c0 = g * GRP_COLS
                    for j0 in range(0, gw, CHUNK):
                        j1 = min(j0 + CHUNK, gw)
                        nc.tensor.matmul(
                            ps[:, j0:j1],
                            q3[:, :, t * QTILE : (t + 1) * QTILE],
                            p3[:, :, c0 + j0 : c0 + j1],
                            start=True,
                            stop=True,
                            perf_mode=DR,
                            skip_group_check=True,
                        )
                    if gtype == "D":
                        wt = wbandp.tile([QTILE, gw // 8], bf16, tag="wt")
                        nc.vector.tensor_reduce(
                            out=wt[:],
                            in_=ps[:].rearrange("p (b x) -> p b x", x=8),
                            axis=mybir.AxisListType.X,
                            op=MAXOP,
                        )
                        nc.sync.dma_start(
                            wb_d[
                                t * QTILE : (t + 1) * QTILE,
                                g * 128 : g * 128 + gw // 8,
                            ],
                            wt[:],
                        )
                    else:
                        stg = stgp.tile([QTILE, GRP_COLS], bf16, tag="stg")
                        nc.scalar.copy(out=stg[:, :gw], in_=ps[:])
                        r = r_of[g]
                        nc.gpsimd.dma_start(
                            raw_d[
                                t * QTILE : (t + 1) * QTILE,
                                r * GRP_COLS : r * GRP_COLS + gw,
                            ],
                            stg[:, :gw],
                        )

    nc.compile()
    return nc


def _prepare(query_emb, query_time, pool_emb, pool_time):
    """Host: fold norms+decay+scale into fp8 operands, sort, shard, scatter."""
    import concourse.mybir as mybir

    f8np = mybir.dt.np(mybir.dt.float8e4)

    q = query_emb.astype(np.float64)
    p = pool_emb.astype(np.float64)
    qt = query_time.astype(np.float64)
    pt = pool_time.astype(np.float64)

    qnorm = np.linalg.norm(q, axis=1)
    pnorm = np.linalg.norm(p, axis=1)
    qs = (q / np.maximum(qnorm, EPS)[:, None]) * (
        SCALE * np.exp(-LAMBDA * qt)[:, None]
    )
    ps = (p / np.maximum(pnorm, EPS)[:, None]) * (
        SCALE * np.exp(LAMBDA * pt)[:, None]
    )

    pperm = np.argsort(pool_time, kind="stable")
    qperm = np.argsort(query_time, kind="stable")
    ps_sorted = ps[pperm]
    pt_sorted = pool_time[pperm]
    qs_sorted = qs[qperm]

    # all group types use the identity layout (time-rank order)
    scatter = np.arange(SHARD, dtype=np.int64)

    # fp8 operand layout [128, 2, N]: partition = h % 128, ksub = h // 128
    def to_dev(x_sorted):  # [N, H] scaled floats -> [128, 2*N] fp8
        xT = np.ascontiguousarray(x_sorted.T.astype(np.float32))  # [256, N]
        x8 = xT.astype(f8np)  # quantize
        n = x8.shape[1]
        return np.ascontiguousarray(
            x8.reshape(2, 128, n).transpose(1, 0, 2).reshape(128, 2 * n)
        )

    shard_emb = []
    shard_times = []
    for k in range(NCORES):
        sh = ps_sorted[k::NCORES]  # [8192, 256] in time order
        shard_emb.append(to_dev(sh[scatter]))
        shard_times.append(pt_sorted[k::NCORES])
    q_dev = to_dev(qs_sorted)

    ci = np.stack(
        [np.searchsorted(shard_times[k], query_time[qperm], side="left")
         for k in range(NCORES)]
    ).astype(np.int64)  # [8, 2048]

    return q_dev, shard_emb, ci, pperm, qperm


def _pt_list(ci):
    ci_tiles = ci.reshape(NCORES, NTILES, QTILE)
    maxci = ci_tiles.max(axis=0).max(axis=1)  # [NTILES]
    return [
        int(min(max(64, int(np.ceil(float(m) / 8.0)) * 8), SHARD))
        for m in maxci
    ]


def _core_in_map(q_dev, shard_emb, k):
    return {"q8": q_dev, "p8": shard_emb[k]}


def _device_windows(q_dev, shard_emb, ci):
    from concourse.bass_utils import run_bass_kernel_spmd

    pt_list = _pt_list(ci)
    key = tuple(pt_list)
    if key not in _PROGRAM_CACHE:
        _PROGRAM_CACHE.clear()
        _PROGRAM_CACHE[key] = _build_program(pt_list)
    nc = _PROGRAM_CACHE[key]

    in_maps = [_core_in_map(q_dev, shard_emb, k) for k in range(NCORES)]
    res = run_bass_kernel_spmd(nc, in_maps, core_ids=list(range(NCORES)))
    wb = np.stack(
        [res.results[k]["wb"].astype(np.float32) for k in range(NCORES)]
    )  # [8, 2048, 1024]
    raw = np.stack(
        [res.results[k]["raw"].astype(np.float32) for k in range(NCORES)]
    )  # [8, 2048, NR*1024]
    _merge_raw_windows(wb, raw, pt_list)
    return wb, pt_list


def _merge_raw_windows(wb, raw, pt_list):
    """Host-side window max for R groups: raw regions (per-tile R-rank
    layout) -> wb window slots."""
    tmap = _type_map(pt_list)
    for t in range(NTILES):
        pt_len = pt_list[t]
        types = tmap[t]
        rows = slice(t * QTILE, (t + 1) * QTILE)
        r = 0
        for g, ty in enumerate(types):
            if ty != "R":
                continue
            gw = min(GRP_COLS, pt_len - g * GRP_COLS)
            blk = raw[:, rows, r * GRP_COLS : r * GRP_COLS + gw]
            wm = blk.reshape(NCORES, QTILE, gw // 8, 8).max(axis=3)
            wb[:, rows, g * 128 : g * 128 + gw // 8] = wm
            r += 1


def _merge_and_score(
    wb, pt_list, ci, pperm, qperm, query_emb, query_time, pool_emb, pool_time
):
    """Select candidate windows by global threshold, rescore exactly, score.

    wb values are sims scaled by SCALE^2."""
    nq = BQ
    wmin = 8 * np.arange(NWIN_MAX, dtype=np.int64)

    nwin_row = (np.asarray(pt_list, dtype=np.int64) // 8)[
        np.repeat(np.arange(NTILES), QTILE)
    ]
    exists = np.arange(NWIN_MAX)[None, :] < nwin_row[:, None]
    wbm = np.where(
        exists[None, :, :] & (wmin[None, None, :] < ci[:, :, None]),
        wb,
        -np.inf,
    )

    flat = np.transpose(wbm, (1, 0, 2)).reshape(nq, NCORES * NWIN_MAX)
    KM = K + MARGIN
    kth = np.partition(flat, -KM, axis=1)[:, -KM]
    kth = kth - (np.abs(kth) * 2.0**-7 + RELAX_ABS)
    thr = np.where(np.isfinite(kth), kth, -1.0e38)
    sel = flat >= thr[:, None]
    nsel = sel.sum(axis=1)

    rows, wcols = np.nonzero(sel)
    core = wcols // NWIN_MAX
    w = wcols % NWIN_MAX
    cols_shard = (8 * w)[:, None] + np.arange(8)[None, :]
    sorted_pos = cols_shard * NCORES + core[:, None]
    orig = pperm[sorted_pos]

    q64 = query_emb.astype(np.float64)
    qn64 = q64 / np.maximum(np.linalg.norm(q64, axis=1), EPS)[:, None]
    qn32 = qn64.astype(np.float32)
    pnorm = np.linalg.norm(pool_emb.astype(np.float64), axis=1)
    pn_inv32 = (1.0 / np.maximum(pnorm, EPS)).astype(np.float32)
    oi_rows = qperm[rows]
    n_ent = rows.shape[0]
    sims = np.empty((n_ent, 8), dtype=np.float64)
    causal = np.empty((n_ent, 8), dtype=bool)
    BLK = 16384
    for b in range(0, n_ent, BLK):
        sl = slice(b, b + BLK)
        emb = pool_emb[orig[sl]]  # [blk, 8, 256] f32
        dots = np.matmul(emb, qn32[oi_rows[sl]][:, :, None])[:, :, 0]
        dots = dots.astype(np.float64) * pn_inv32[orig[sl]].astype(np.float64)
        tdiff = np.abs(
            query_time[oi_rows[sl]].astype(np.float64)[:, None]
            - pool_time[orig[sl]].astype(np.float64)
        )
        sims[sl] = dots * np.exp(-LAMBDA * tdiff)
        causal[sl] = pool_time[orig[sl]] < query_time[oi_rows[sl]][:, None]

    maxw = max(1, min(int(nsel.max()) if n_ent else 1, MAXW_ROW))
    slot = np.zeros(n_ent, dtype=np.int64)
    if n_ent:
        row_start = np.searchsorted(rows, np.arange(nq), side="left")
        slot = np.arange(n_ent) - row_start[rows]
    keep = slot < MAXW_ROW
    dsims = np.full((nq, maxw * 8), -np.inf)
    dorig = np.zeros((nq, maxw * 8), dtype=np.int64)
    rk = rows[keep]
    sk = slot[keep]
    for o in range(8):
        dsims[rk, sk * 8 + o] = np.where(causal[keep, o], sims[keep, o], -np.inf)
        dorig[rk, sk * 8 + o] = orig[keep, o]

    order2 = np.lexsort((dorig, -dsims), axis=1)[:, :K]
    top_idx = np.take_along_axis(dorig, order2, axis=1)
    nvalid_row = np.isfinite(np.take_along_axis(dsims, order2, axis=1)).sum(axis=1)

    pt_min = pool_time.min()
    n_causal_global = np.searchsorted(
        np.sort(pool_time), query_time[qperm], side="left"
    )
    fix_rows = np.nonzero(
        (query_time[qperm] <= pt_min)
        | (np.minimum(n_causal_global, K) > nvalid_row)
        | (n_causal_global < K)
        | (nsel > MAXW_ROW)
    )[0]
    if len(fix_rows):
        pn_all = pool_emb.astype(np.float64) / np.maximum(pnorm, EPS)[:, None]
    for i in fix_rows:
        oi = qperm[i]
        ti = query_time[oi]
        sims_all = (pn_all @ qn64[oi]) * np.exp(
            -LAMBDA * np.abs(float(ti) - pool_time.astype(np.float64))
        )
        if ti <= pt_min:
            top_idx[i] = np.argsort(-sims_all, kind="stable")[:K]
            continue
        causal_all = pool_time < ti
        c = int(causal_all.sum())
        masked_all = np.where(causal_all, sims_all, -np.inf)
        picks = list(np.argsort(-masked_all, kind="stable")[: min(c, K)])
        j = 0
        while len(picks) < K:
            if not causal_all[j]:
                picks.append(j)
            j += 1
        top_idx[i] = np.array(picks, dtype=np.int64)

    q = query_emb.astype(np.float64)[qperm]
    retrieved = pool_emb.astype(np.float64)[top_idx]
    scale = float(H) ** -0.5
    logits = np.einsum("bh,bkh->bk", q, retrieved) * scale
    logits -= logits.max(axis=1, keepdims=True)
    e = np.exp(logits)
    attn = e / e.sum(axis=1, keepdims=True)
    fused = np.einsum("bk,bkh->bh", attn, retrieved)

    qn2 = np.linalg.norm(q, axis=1)
    fn2 = np.linalg.norm(fused, axis=1)
    cos = np.sum(q * fused, axis=1) / np.maximum(qn2 * fn2, COS_EPS)
    l2 = np.linalg.norm(q - fused, axis=1)
    score_sorted = GAMMA * (1.0 - cos) + DELTA * l2

    out = np.zeros(nq, dtype=np.float32)
    out[qperm] = score_sorted.astype(np.float32)
    return out


def kernel(query_emb, query_time, pool_emb, pool_time):
    query_emb = np.asarray(query_emb, dtype=np.float32)
    query_time = np.asarray(query_time, dtype=np.float32)
    pool_emb = np.asarray(pool_emb, dtype=np.float32)
    pool_time = np.asarray(pool_time, dtype=np.float32)

    q_dev, shard_emb, ci, pperm, qperm = _prepare(
        query_emb, query_time, pool_emb, pool_time
    )
    wb, pt_list = _device_windows(q_dev, shard_emb, ci)
    return _merge_and_score(
        wb, pt_list, ci, pperm, qperm, query_emb, query_time, pool_emb, pool_time
    )
